# revision 7
# baseline (speedup 1.0000x reference)
"""CTC forward-loss kernel for Trainium2, 8 cores data-parallel (32 batch/core).

v2 architecture (validated against a numpy prototype):
  Layout: 128 partitions = 4 t-chunks x 32 batches; free axis = t within
  chunk (500).  Skew-2 wavefront over columns s: chunk c scans column
  s = sigma - 2c at step sigma (507 steps).

  Per sigma:
    PE    4 select-matmuls W_s.T @ X_c -> PSUM produce the move coefficients
          b[t] = E[t, seqs[s-1]] for each chunk (one-hot W, fp16 tables),
          issued 2 sigmas ahead; plus a tiny halo-shift matmul.
    DVE   u = (b * d) * prv   (scalar_tensor_tensor from PSUM)
          cur[1:] = scan(d * y_prev + u)  (tensor_tensor_scan, fp32 state)
    ACT   init-slot copy (halo -> cur[:,0]) and chunk-3 output staging.

  Numerics: stored values are G * exp(-beta_c(epoch) * t_local) * renorm,
  with a compile-time beta schedule (binomial-slope model) and a shared
  per-batch renorm every R=8 sigmas measured from the full-buffer sum.
  Epoch transitions re-ramp the state on device (ACT exp of iota).  The
  host replays the schedule + logged renorm factors to undo all scales.
"""

import math
import numpy as np

import concourse.bass as bass
import concourse.tile as tile
import concourse.mybir as mybir
from concourse.bass_utils import run_bass_kernel_spmd

NT, NB, NF, NS = 2000, 256, 5, 500
M = 8
B = NB // M           # 32 local batches
NC, TC = 4, 500
SKEW = 2
SIG = NS + SKEW * (NC - 1) + 1          # 507 wavefront steps
R = 8                                    # renorm/epoch cadence
NEP = 80                                 # schedule table width (>= SIG//R + 2)
LA = 2                                   # select lookahead (psum bufs = 4)
ZTINY = 1e-30
F32 = mybir.dt.float32
F16 = mybir.dt.float16
AL = mybir.AluOpType
AF = mybir.ActivationFunctionType
AX = mybir.AxisListType

_CACHE = {}


# ---------------------------------------------------------------- schedule --

def _lnC(n, k):
    if k < 0 or k > n:
        return -math.inf
    return (math.lgamma(n + 1) - math.lgamma(k + 1) - math.lgamma(n - k + 1))


def _beta_model(c, s):
    t0, t1 = 500 * c, 500 * (c + 1)
    s = min(s, t1 - 64)
    tlo = max(t0, s + 1)
    if t1 <= tlo + 1:
        return 0.0
    num = _lnC(t1, s) - _lnC(tlo, s)
    if not math.isfinite(num):
        return 0.0
    return num / (t1 - tlo)


def beta_schedule():
    """BETA[c, ep], monotone non-decreasing per chunk."""
    n_ep = NEP
    BETA = np.zeros((NC, n_ep))
    for c in range(NC):
        prev = 0.0
        for ep in range(n_ep):
            smid = min(max(ep * R + R // 2 - SKEW * c, 0), NS)
            b = max(_beta_model(c, smid), prev)
            BETA[c, ep] = b
            prev = b
    return BETA


# ---------------------------------------------------------------- module ----

def _split_multi_waits(nc, max_inline=1):
    """walrus allows few fused sem-waits per instruction; hoist extras onto
    EventSemaphore carriers on the same engine stream."""
    f = nc.m.functions[0]
    for bb in f.blocks:
        out = []
        changed = False
        for inst in bb.instructions:
            si = inst.sync_info
            waits = list(si.on_wait) if si is not None and si.on_wait else []
            if isinstance(inst, mybir.InstEventSemaphore) and len(waits) > 2:
                keep, extra = waits[:2], waits[2:]
            elif not isinstance(inst, mybir.InstEventSemaphore) \
                    and len(waits) > max_inline:
                keep, extra = waits[:max_inline], waits[max_inline:]
            else:
                keep, extra = waits, []
            if extra:
                for j in range(0, len(extra), 2):
                    evt = mybir.InstEventSemaphore(
                        name=f"evw{j}_{inst.name}", ins=[], outs=[])
                    evt.engine = inst.engine
                    evt.sync_info = mybir.SyncInfo(
                        on_wait=extra[j:j + 2], on_update=[])
                    out.append(evt)
                inst.sync_info = mybir.SyncInfo(
                    on_wait=keep, on_update=list(si.on_update))
                changed = True
            out.append(inst)
        if changed:
            bb.instructions = out


def build_module(split_waits=True):
    key = ("nc", split_waits)
    if key in _CACHE:
        return _CACHE[key], _CACHE["names"]
    nc = bass.Bass(debug=False)

    xtab_d = nc.dram_tensor("xtab", [128, NC * TC], F16, kind="ExternalInput")
    wtab_d = nc.dram_tensor("wtab", [128, (NS + 1) * 32], F16, kind="ExternalInput")
    mats_d = nc.dram_tensor("mats", [128, 256], F32, kind="ExternalInput")
    sched_d = nc.dram_tensor("sched", [128, 4 * NEP], F32, kind="ExternalInput")
    iot_d = nc.dram_tensor("iot", [128, 512], F32, kind="ExternalInput")
    out_d = nc.dram_tensor("outst", [32, 512], F32, kind="ExternalOutput")
    rz_d = nc.dram_tensor("rzst", [128, NEP], F32, kind="ExternalOutput")

    with tile.TileContext(nc) as tc, \
         tc.tile_pool(name="state", bufs=1) as st:
        xtab = st.tile([128, NC * TC], F16, tag="xtab", name="xtab_sb")
        wtab = st.tile([128, (NS + 1) * 32], F16, tag="wtab", name="wtab_sb")
        mats = st.tile([128, 256], F32, tag="mats", name="mats_sb")
        sched = st.tile([128, 4 * NEP], F32, tag="sched", name="sched_sb")
        iot = st.tile([128, 512], F32, tag="iot", name="iot_sb")
        bufA = st.tile([128, TC + 1], F32, tag="bufA", name="bufA")
        bufB = st.tile([128, TC + 1], F32, tag="bufB", name="bufB")
        u = st.tile([128, TC + 1], F32, tag="u", name="u")
        d0row = st.tile([128, TC], F32, tag="d0row", name="d0row")
        ramp = st.tile([128, TC + 1], F32, tag="ramp", name="ramp")
        ones = st.tile([128, TC + 1], F32, tag="ones", name="ones")
        outst = st.tile([128, 512], F32, tag="outst", name="outst")
        rzst = st.tile([128, NEP], F32, tag="rzst", name="rzst")
        zred = st.tile([128, 1], F32, tag="zred", name="zred")
        zcl = st.tile([128, 1], F32, tag="zcl", name="zcl")
        haloSB = st.tile([128, 2], F32, tag="haloSB", name="haloSB")
        bsb = [st.tile([128, TC], F32, tag=f"bsb{i}", name=f"bsb{i}")
               for i in range(3)]
        rz = st.tile([128, 1], F32, tag="rz", name="rz")
        vsc = st.tile([128, 1], F32, tag="vsc", name="vsc")

        nc.sync.dma_start(out=xtab[:, :], in_=xtab_d.ap()[:, :])
        nc.sync.dma_start(out=wtab[:, 0:2048], in_=wtab_d.ap()[:, 0:2048])
        nc.sync.dma_start(out=wtab[:, 2048:], in_=wtab_d.ap()[:, 2048:])
        nc.sync.dma_start(out=mats[:, :], in_=mats_d.ap()[:, :])
        nc.sync.dma_start(out=sched[:, :], in_=sched_d.ap()[:, :])
        nc.sync.dma_start(out=iot[:, :], in_=iot_d.ap()[:, :])

        DT = sched[:, 0 * NEP:1 * NEP]       # exp(-beta_c(ep))
        DBN = sched[:, 1 * NEP:2 * NEP]      # -(beta_c(ep) - beta_c(ep-1))
        DBI = sched[:, 2 * NEP:3 * NEP]      # -TC * sum_{c'<c} dbeta_{c'}(ep)
        FXB = sched[:, 3 * NEP:4 * NEP]      # exp(-TC*sum_{c'<=c-1} dbeta(ep))
        PERM = mats[:, 0:128]
        GSUM = mats[:, 128:256]

        for t in (bufA, bufB, u, outst, rzst, haloSB):
            nc.vector.memset(t[:, :], 0.0)
        nc.vector.memset(ones[:, :], 1.0)
        # touch DMA'd tiles once (wait-slot pressure)
        scr = st.tile([128, 1], F32, tag="scr", name="scr")
        nc.vector.tensor_copy(scr[:, 0:1], mats[:, 0:1])
        nc.vector.tensor_copy(scr[:, 0:1], sched[:, 0:1])
        nc.vector.tensor_copy(scr[:, 0:1], iot[:, 0:1])

        # initial d0row for epoch 0
        nc.scalar.mul(d0row[:, 0:TC], ones[:, 0:TC], DT[:, 0:1])

        with tc.tile_pool(name="bsel", bufs=4, space="PSUM") as bp, \
             tc.tile_pool(name="zs", bufs=2, space="PSUM") as zp:
            bps = [None] * (SIG + LA)

            def issue_selects(sL):
                bt = bp.tile([128, 512], F32, tag="bsel")
                bps[sL] = bt
                for c in range(NC):
                    s = sL - SKEW * c
                    slot = s if 1 <= s <= NS else 0
                    nc.tensor.matmul(
                        bt[32 * c:32 * c + 32, 0:TC],
                        wtab[:, 32 * slot:32 * slot + 32],
                        xtab[:, TC * c:TC * (c + 1)],
                        start=True, stop=True, tile_position=(0, 32 * c))

            for sL in range(LA):
                issue_selects(sL)

            for sig in range(SIG):
                cur, prv = (bufA, bufB) if sig % 2 == 0 else (bufB, bufA)
                ep = sig // R
                event = (sig % R == 0 and sig > 0)
                if event:
                    # epoch ramp: exp(-dbeta*j - TC*cumdbeta)
                    nc.scalar.activation(ramp[:, 0:TC + 1], iot[:, 0:TC + 1],
                                         AF.Exp, scale=DBN[:, ep:ep + 1],
                                         bias=DBI[:, ep:ep + 1])
                    nc.vector.tensor_mul(prv[:, 0:TC + 1], prv[:, 0:TC + 1],
                                         ramp[:, 0:TC + 1])
                    # shared renorm from full-buffer sum
                    nc.vector.tensor_reduce(zred[:, 0:1],
                                            prv[:, 0:TC + 1:4],
                                            AX.X, AL.add)
                    zsp = zp.tile([128, 1], F32, tag="zs")
                    nc.tensor.matmul(zsp[:, 0:1], GSUM, zred[:, 0:1],
                                     start=True, stop=True)
                    nc.vector.tensor_scalar_max(zcl[:, 0:1], zsp[:, 0:1], ZTINY)
                    nc.vector.reciprocal(rz[:, 0:1], zcl[:, 0:1])
                    nc.scalar.mul(prv[:, 0:TC + 1], prv[:, 0:TC + 1], rz[:, 0:1])
                    nc.scalar.copy(rzst[:, ep:ep + 1], rz[:, 0:1])
                    nc.scalar.mul(vsc[:, 0:1], FXB[:, ep:ep + 1], rz[:, 0:1])
                    # refresh drain row for the new epoch
                    nc.scalar.mul(d0row[:, 0:TC], ones[:, 0:TC], DT[:, ep:ep + 1])

                # select lookahead
                if sig + LA < SIG:
                    issue_selects(sig + LA)
                # stage b for sig+1 from PSUM into SBUF (ACT, off critical path)
                if sig + 1 < SIG:
                    nc.scalar.copy(bsb[(sig + 1) % 3][:, 0:TC],
                                   bps[sig + 1][:, 0:TC])

                # init slot: halo (written at sig-2) with event fixes
                if sig == 0:
                    nc.vector.memset(cur[:, 0:1], 0.0)
                    nc.vector.memset(cur[0:32, 0:1], 1.0)
                else:
                    hsrc = haloSB[:, sig % 2:sig % 2 + 1]
                    if sig % R in (0, 1) and sig >= R:
                        nc.scalar.activation(cur[:, 0:1], hsrc, AF.Copy,
                                             scale=vsc[:, 0:1])
                    else:
                        nc.scalar.activation(cur[:, 0:1], hsrc, AF.Copy)

                # u = (b * d) * prv ; slot 0 of u stays 0
                bsrc = bsb[sig % 3][:, 0:TC] if sig > 0 else bps[0][:, 0:TC]
                nc.vector.scalar_tensor_tensor(
                    u[:, 0:TC], bsrc, DT[:, ep:ep + 1],
                    prv[:, 0:TC], AL.mult, AL.mult)
                # scan
                nc.vector.tensor_tensor_scan(
                    cur[:, 1:TC + 1], d0row[:, 0:TC], u[:, 0:TC],
                    cur[:, 0:1], AL.mult, AL.add)

                # halo extraction for sig+2 (partition-shift by +32 via DMA)
                if sig + 2 < SIG:
                    nc.sync.dma_start(
                        out=haloSB[32:128, sig % 2:sig % 2 + 1],
                        in_=cur[0:96, TC:TC + 1])

                # chunk-3 output staging
                s3 = sig - SKEW * (NC - 1)
                if 0 <= s3 <= NS:
                    nc.scalar.copy(outst[96:128, s3:s3 + 1],
                                   cur[96:128, TC:TC + 1])

        nc.sync.dma_start(out=out_d.ap()[:, :], in_=outst[96:128, 0:512])
        nc.sync.dma_start(out=rz_d.ap()[:, :], in_=rzst[:, :])

    if split_waits:
        _split_multi_waits(nc)

    _CACHE[key] = nc
    _CACHE["names"] = dict(ins=["xtab", "wtab", "mats", "sched", "iot"],
                           out=["outst", "rzst"])
    return nc, _CACHE["names"]


# ---------------------------------------------------------------- host ------

def host_prep(x, seqs):
    f32, f16 = np.float32, np.float16
    BETA = beta_schedule()
    x = np.asarray(x)
    seqs = np.asarray(seqs)

    # schedule tables, per partition p = 32c + b
    dt_t = np.zeros((128, NEP), f32)
    dbn_t = np.zeros((128, NEP), f32)
    dbi_t = np.zeros((128, NEP), f32)
    fxb_t = np.ones((128, NEP), f32)
    for ep in range(NEP):
        db = BETA[:, ep] - (BETA[:, ep - 1] if ep > 0 else BETA[:, 0])
        cum = np.concatenate([[0.0], np.cumsum(db)[:-1]])    # sum_{c'<c}
        dsum = np.cumsum(db)                                  # sum_{c'<=c}
        for c in range(NC):
            p = slice(32 * c, 32 * c + 32)
            dt_t[p, ep] = np.exp(-BETA[c, ep])
            dbn_t[p, ep] = -db[c]
            dbi_t[p, ep] = -TC * cum[c]
            fxb_t[p, ep] = np.exp(-TC * dsum[c - 1]) if c >= 1 else 1.0
    sched = np.concatenate([dt_t, dbn_t, dbi_t, fxb_t], axis=1)

    iot = np.broadcast_to(np.arange(512, dtype=f32), (128, 512)).copy()

    mats = np.zeros((128, 256), f32)
    for q in range(96):
        mats[q, 32 + q] = 1.0            # perm: out[p] = in[p-32]
    for pi in range(128):
        for c in range(NC):
            mats[pi, 128 + ((pi % 32) + 32 * c) % 128] = 0.0  # placeholder
    # gsum: out[po] = sum_c in[32c + (po%32)]
    gs = np.zeros((128, 128), f32)
    for po in range(128):
        for c in range(NC):
            gs[32 * c + (po % 32), po] = 1.0
    mats[:, 128:256] = gs

    in_maps = []
    for m in range(M):
        xb = x[:, B * m:B * (m + 1), :].astype(f32)           # (NT, 32, 5)
        E = np.exp(xb[:, :, :4] - xb[:, :, 4:5])              # (NT, 32, 4)
        xt = np.empty((128, NC * TC), f16)
        for c in range(NC):
            blk = E[500 * c:500 * (c + 1)]                    # (500, 32, 4)
            for k in range(4):
                xt[32 * k:32 * k + 32, TC * c:TC * (c + 1)] = \
                    blk[:, :, k].T.astype(f16)
        sq = seqs[B * m:B * (m + 1)]                          # (32, NS)
        wt = np.zeros((128, (NS + 1) * 32), f16)
        for s in range(1, NS + 1):
            k = sq[:, s - 1]                                  # (32,)
            wt[32 * k + np.arange(B), 32 * s + np.arange(B)] = 1.0
        in_maps.append({"xtab": xt, "wtab": wt, "mats": mats,
                        "sched": sched, "iot": iot})
    return in_maps


def host_post(x, seqlens, outs, rzs):
    BETA = beta_schedule()
    x = np.asarray(x)
    seqlens = np.asarray(seqlens)
    loss = np.zeros((NB, 1), np.float32)
    for m in range(M):
        raw = outs[m][:, 0:NS + 1].astype(np.float64)         # (32, 501)
        rzv = rzs[m][0:32, :].astype(np.float64)              # rz per batch,epoch
        lnrz = np.zeros((32, SIG))
        for ep in range(1, NEP):
            sg = ep * R
            if sg < SIG:
                lnrz[:, sg] = np.log(np.maximum(rzv[:, ep], 1e-300))
        lnrz_cum = np.cumsum(lnrz, axis=1)
        C2000 = x[:, B * m:B * (m + 1), 4].sum(axis=0, dtype=np.float64)
        fwd = np.zeros((32, NS + 1))
        for s in range(NS + 1):
            sig = s + SKEW * (NC - 1)
            ep = sig // R
            corr = TC * BETA[:, ep].sum() - lnrz_cum[:, sig]
            fwd[:, s] = np.log(np.maximum(raw[:, s], 1e-300)) + corr + C2000
        sl = seqlens[B * m:B * (m + 1)].astype(np.int64)
        loss[B * m:B * (m + 1), 0] = \
            (-np.take_along_axis(fwd, sl[:, None], axis=1) / NT)[:, 0]
    return loss.astype(np.float32)


def kernel(x, seqs, seqlens):
    nc, names = build_module()
    in_maps = host_prep(x, seqs)
    res = run_bass_kernel_spmd(nc, in_maps, list(range(M)))
    outs = [res.results[m]["outst"] for m in range(M)]
    rzs = [res.results[m]["rzst"] for m in range(M)]
    return host_post(x, seqlens, outs, rzs)


# revision 8
# speedup vs baseline: 1.0824x; 1.0824x over previous
"""CTC forward-loss kernel for Trainium2, 8 cores data-parallel (32 batch/core).

v2 architecture (validated against a numpy prototype):
  Layout: 128 partitions = 4 t-chunks x 32 batches; free axis = t within
  chunk (500).  Skew-2 wavefront over columns s: chunk c scans column
  s = sigma - 2c at step sigma (507 steps).

  Per sigma:
    PE    4 select-matmuls W_s.T @ X_c -> PSUM produce the move coefficients
          b[t] = E[t, seqs[s-1]] for each chunk (one-hot W, fp16 tables),
          issued 2 sigmas ahead; plus a tiny halo-shift matmul.
    DVE   u = (b * d) * prv   (scalar_tensor_tensor from PSUM)
          cur[1:] = scan(d * y_prev + u)  (tensor_tensor_scan, fp32 state)
    ACT   init-slot copy (halo -> cur[:,0]) and chunk-3 output staging.

  Numerics: stored values are G * exp(-beta_c(epoch) * t_local) * renorm,
  with a compile-time beta schedule (binomial-slope model) and a shared
  per-batch renorm every R=8 sigmas measured from the full-buffer sum.
  Epoch transitions re-ramp the state on device (ACT exp of iota).  The
  host replays the schedule + logged renorm factors to undo all scales.
"""

import math
import numpy as np

import concourse.bass as bass
import concourse.tile as tile
import concourse.mybir as mybir
from concourse.bass_utils import run_bass_kernel_spmd

NT, NB, NF, NS = 2000, 256, 5, 500
M = 8
B = NB // M           # 32 local batches
NC, TC = 4, 500
SKEW = 2
SIG = NS + SKEW * (NC - 1) + 1          # 507 wavefront steps
R = 8                                    # renorm/epoch cadence
NEP = 80                                 # schedule table width (>= SIG//R + 2)
LA = 2                                   # select lookahead (psum bufs = 4)
ZTINY = 1e-30
F32 = mybir.dt.float32
F16 = mybir.dt.float16
AL = mybir.AluOpType
AF = mybir.ActivationFunctionType
AX = mybir.AxisListType

_CACHE = {}


# ---------------------------------------------------------------- schedule --

def _lnC(n, k):
    if k < 0 or k > n:
        return -math.inf
    return (math.lgamma(n + 1) - math.lgamma(k + 1) - math.lgamma(n - k + 1))


def _beta_model(c, s):
    t0, t1 = 500 * c, 500 * (c + 1)
    s = min(s, t1 - 64)
    tlo = max(t0, s + 1)
    if t1 <= tlo + 1:
        return 0.0
    num = _lnC(t1, s) - _lnC(tlo, s)
    if not math.isfinite(num):
        return 0.0
    return num / (t1 - tlo)


def beta_schedule():
    """BETA[c, ep], monotone non-decreasing per chunk."""
    n_ep = NEP
    BETA = np.zeros((NC, n_ep))
    for c in range(NC):
        prev = 0.0
        for ep in range(n_ep):
            smid = min(max(ep * R + R // 2 - SKEW * c, 0), NS)
            b = max(_beta_model(c, smid), prev)
            BETA[c, ep] = b
            prev = b
    return BETA


# ---------------------------------------------------------------- module ----

def _split_multi_waits(nc, max_inline=1):
    """walrus allows few fused sem-waits per instruction; hoist extras onto
    EventSemaphore carriers on the same engine stream."""
    f = nc.m.functions[0]
    for bb in f.blocks:
        out = []
        changed = False
        for inst in bb.instructions:
            si = inst.sync_info
            waits = list(si.on_wait) if si is not None and si.on_wait else []
            if isinstance(inst, mybir.InstEventSemaphore) and len(waits) > 2:
                keep, extra = waits[:2], waits[2:]
            elif not isinstance(inst, mybir.InstEventSemaphore) \
                    and len(waits) > max_inline:
                keep, extra = waits[:max_inline], waits[max_inline:]
            else:
                keep, extra = waits, []
            if extra:
                for j in range(0, len(extra), 2):
                    evt = mybir.InstEventSemaphore(
                        name=f"evw{j}_{inst.name}", ins=[], outs=[])
                    evt.engine = inst.engine
                    evt.sync_info = mybir.SyncInfo(
                        on_wait=extra[j:j + 2], on_update=[])
                    out.append(evt)
                inst.sync_info = mybir.SyncInfo(
                    on_wait=keep, on_update=list(si.on_update))
                changed = True
            out.append(inst)
        if changed:
            bb.instructions = out


def build_module(split_waits=True):
    key = ("nc", split_waits)
    if key in _CACHE:
        return _CACHE[key], _CACHE["names"]
    nc = bass.Bass(debug=False)

    xtab_d = nc.dram_tensor("xtab", [128, NC * TC], F16, kind="ExternalInput")
    wtab_d = nc.dram_tensor("wtab", [128, (NS + 1) * 32], F16, kind="ExternalInput")
    mats_d = nc.dram_tensor("mats", [128, 256], F32, kind="ExternalInput")
    sched_d = nc.dram_tensor("sched", [128, 4 * NEP], F32, kind="ExternalInput")
    iot_d = nc.dram_tensor("iot", [128, 512], F32, kind="ExternalInput")
    out_d = nc.dram_tensor("outst", [32, 512], F32, kind="ExternalOutput")
    rz_d = nc.dram_tensor("rzst", [128, NEP], F32, kind="ExternalOutput")

    with tile.TileContext(nc) as tc, \
         tc.tile_pool(name="state", bufs=1) as st:
        xtab = st.tile([128, NC * TC], F16, tag="xtab", name="xtab_sb")
        wtab = st.tile([128, (NS + 1) * 32], F16, tag="wtab", name="wtab_sb")
        mats = st.tile([128, 256], F32, tag="mats", name="mats_sb")
        sched = st.tile([128, 4 * NEP], F32, tag="sched", name="sched_sb")
        iot = st.tile([128, 512], F32, tag="iot", name="iot_sb")
        bufA = st.tile([128, TC + 1], F32, tag="bufA", name="bufA")
        bufB = st.tile([128, TC + 1], F32, tag="bufB", name="bufB")
        u = st.tile([128, TC + 1], F32, tag="u", name="u")
        d0row = st.tile([128, TC], F32, tag="d0row", name="d0row")
        ramp = st.tile([128, TC + 1], F32, tag="ramp", name="ramp")
        ones = st.tile([128, TC + 1], F32, tag="ones", name="ones")
        outst = st.tile([128, 512], F32, tag="outst", name="outst")
        rzst = st.tile([128, NEP], F32, tag="rzst", name="rzst")
        zred = st.tile([128, 1], F32, tag="zred", name="zred")
        zcl = st.tile([128, 1], F32, tag="zcl", name="zcl")
        haloSB = st.tile([128, 2], F32, tag="haloSB", name="haloSB")

        rz = st.tile([128, 1], F32, tag="rz", name="rz")
        vsc = st.tile([128, 1], F32, tag="vsc", name="vsc")

        nc.sync.dma_start(out=xtab[:, :], in_=xtab_d.ap()[:, :])
        nc.sync.dma_start(out=wtab[:, 0:2048], in_=wtab_d.ap()[:, 0:2048])
        nc.sync.dma_start(out=wtab[:, 2048:], in_=wtab_d.ap()[:, 2048:])
        nc.sync.dma_start(out=mats[:, :], in_=mats_d.ap()[:, :])
        nc.sync.dma_start(out=sched[:, :], in_=sched_d.ap()[:, :])
        nc.sync.dma_start(out=iot[:, :], in_=iot_d.ap()[:, :])

        DT = sched[:, 0 * NEP:1 * NEP]       # exp(-beta_c(ep))
        DBN = sched[:, 1 * NEP:2 * NEP]      # -(beta_c(ep) - beta_c(ep-1))
        DBI = sched[:, 2 * NEP:3 * NEP]      # -TC * sum_{c'<c} dbeta_{c'}(ep)
        FXB = sched[:, 3 * NEP:4 * NEP]      # exp(-TC*sum_{c'<=c-1} dbeta(ep))
        PERM = mats[:, 0:128]
        GSUM = mats[:, 128:256]

        for t in (bufA, bufB, u, outst, rzst, haloSB):
            nc.vector.memset(t[:, :], 0.0)
        nc.vector.memset(ones[:, :], 1.0)
        # touch DMA'd tiles once (wait-slot pressure)
        scr = st.tile([128, 1], F32, tag="scr", name="scr")
        nc.vector.tensor_copy(scr[:, 0:1], mats[:, 0:1])
        nc.vector.tensor_copy(scr[:, 0:1], sched[:, 0:1])
        nc.vector.tensor_copy(scr[:, 0:1], iot[:, 0:1])

        # initial d0row for epoch 0
        nc.scalar.mul(d0row[:, 0:TC], ones[:, 0:TC], DT[:, 0:1])

        with tc.tile_pool(name="bsel", bufs=4, space="PSUM") as bp, \
             tc.tile_pool(name="zs", bufs=2, space="PSUM") as zp:
            bps = [None] * (SIG + LA)

            def issue_selects(sL):
                bt = bp.tile([128, 512], F32, tag="bsel")
                bps[sL] = bt
                for c in range(NC):
                    s = sL - SKEW * c
                    slot = s if 1 <= s <= NS else 0
                    nc.tensor.matmul(
                        bt[32 * c:32 * c + 32, 0:TC],
                        wtab[:, 32 * slot:32 * slot + 32],
                        xtab[:, TC * c:TC * (c + 1)],
                        start=True, stop=True, tile_position=(0, 32 * c))

            for sL in range(LA):
                issue_selects(sL)

            for sig in range(SIG):
                cur, prv = (bufA, bufB) if sig % 2 == 0 else (bufB, bufA)
                ep = sig // R
                event = (sig % R == 0 and sig > 0)
                if event:
                    # epoch ramp: exp(-dbeta*j - TC*cumdbeta)
                    nc.scalar.activation(ramp[:, 0:TC + 1], iot[:, 0:TC + 1],
                                         AF.Exp, scale=DBN[:, ep:ep + 1],
                                         bias=DBI[:, ep:ep + 1])
                    nc.vector.tensor_mul(prv[:, 0:TC + 1], prv[:, 0:TC + 1],
                                         ramp[:, 0:TC + 1])
                    # shared renorm from full-buffer sum
                    nc.vector.tensor_reduce(zred[:, 0:1],
                                            prv[:, 0:TC + 1:4],
                                            AX.X, AL.add)
                    zsp = zp.tile([128, 1], F32, tag="zs")
                    nc.tensor.matmul(zsp[:, 0:1], GSUM, zred[:, 0:1],
                                     start=True, stop=True)
                    nc.vector.tensor_scalar_max(zcl[:, 0:1], zsp[:, 0:1], ZTINY)
                    nc.vector.reciprocal(rz[:, 0:1], zcl[:, 0:1])
                    nc.scalar.mul(prv[:, 0:TC + 1], prv[:, 0:TC + 1], rz[:, 0:1])
                    nc.scalar.copy(rzst[:, ep:ep + 1], rz[:, 0:1])
                    nc.scalar.mul(vsc[:, 0:1], FXB[:, ep:ep + 1], rz[:, 0:1])
                    # refresh drain row for the new epoch
                    nc.scalar.mul(d0row[:, 0:TC], ones[:, 0:TC], DT[:, ep:ep + 1])

                # select lookahead
                if sig + LA < SIG:
                    issue_selects(sig + LA)


                # init slot: halo (written at sig-2) with event fixes
                if sig == 0:
                    nc.vector.memset(cur[:, 0:1], 0.0)
                    nc.vector.memset(cur[0:32, 0:1], 1.0)
                else:
                    hsrc = haloSB[:, sig % 2:sig % 2 + 1]
                    if sig % R in (0, 1) and sig >= R:
                        nc.scalar.activation(cur[:, 0:1], hsrc, AF.Copy,
                                             scale=vsc[:, 0:1])
                    else:
                        nc.scalar.activation(cur[:, 0:1], hsrc, AF.Copy)

                # u = (b * d) * prv ; slot 0 of u stays 0
                nc.vector.scalar_tensor_tensor(
                    u[:, 0:TC], bps[sig][:, 0:TC], DT[:, ep:ep + 1],
                    prv[:, 0:TC], AL.mult, AL.mult)
                # scan
                nc.vector.tensor_tensor_scan(
                    cur[:, 1:TC + 1], d0row[:, 0:TC], u[:, 0:TC],
                    cur[:, 0:1], AL.mult, AL.add)

                # halo extraction for sig+2 (partition-shift by +32 via DMA)
                if sig + 2 < SIG:
                    nc.sync.dma_start(
                        out=haloSB[32:128, sig % 2:sig % 2 + 1],
                        in_=cur[0:96, TC:TC + 1])

                # chunk-3 output staging
                s3 = sig - SKEW * (NC - 1)
                if 0 <= s3 <= NS:
                    nc.scalar.copy(outst[96:128, s3:s3 + 1],
                                   cur[96:128, TC:TC + 1])

        nc.sync.dma_start(out=out_d.ap()[:, :], in_=outst[96:128, 0:512])
        nc.sync.dma_start(out=rz_d.ap()[:, :], in_=rzst[:, :])

    if split_waits:
        _split_multi_waits(nc)

    _CACHE[key] = nc
    _CACHE["names"] = dict(ins=["xtab", "wtab", "mats", "sched", "iot"],
                           out=["outst", "rzst"])
    return nc, _CACHE["names"]


# ---------------------------------------------------------------- host ------

def host_prep(x, seqs):
    f32, f16 = np.float32, np.float16
    BETA = beta_schedule()
    x = np.asarray(x)
    seqs = np.asarray(seqs)

    # schedule tables, per partition p = 32c + b
    dt_t = np.zeros((128, NEP), f32)
    dbn_t = np.zeros((128, NEP), f32)
    dbi_t = np.zeros((128, NEP), f32)
    fxb_t = np.ones((128, NEP), f32)
    for ep in range(NEP):
        db = BETA[:, ep] - (BETA[:, ep - 1] if ep > 0 else BETA[:, 0])
        cum = np.concatenate([[0.0], np.cumsum(db)[:-1]])    # sum_{c'<c}
        dsum = np.cumsum(db)                                  # sum_{c'<=c}
        for c in range(NC):
            p = slice(32 * c, 32 * c + 32)
            dt_t[p, ep] = np.exp(-BETA[c, ep])
            dbn_t[p, ep] = -db[c]
            dbi_t[p, ep] = -TC * cum[c]
            fxb_t[p, ep] = np.exp(-TC * dsum[c - 1]) if c >= 1 else 1.0
    sched = np.concatenate([dt_t, dbn_t, dbi_t, fxb_t], axis=1)

    iot = np.broadcast_to(np.arange(512, dtype=f32), (128, 512)).copy()

    mats = np.zeros((128, 256), f32)
    for q in range(96):
        mats[q, 32 + q] = 1.0            # perm: out[p] = in[p-32]
    for pi in range(128):
        for c in range(NC):
            mats[pi, 128 + ((pi % 32) + 32 * c) % 128] = 0.0  # placeholder
    # gsum: out[po] = sum_c in[32c + (po%32)]
    gs = np.zeros((128, 128), f32)
    for po in range(128):
        for c in range(NC):
            gs[32 * c + (po % 32), po] = 1.0
    mats[:, 128:256] = gs

    in_maps = []
    for m in range(M):
        xb = x[:, B * m:B * (m + 1), :].astype(f32)           # (NT, 32, 5)
        E = np.exp(xb[:, :, :4] - xb[:, :, 4:5])              # (NT, 32, 4)
        xt = np.empty((128, NC * TC), f16)
        for c in range(NC):
            blk = E[500 * c:500 * (c + 1)]                    # (500, 32, 4)
            for k in range(4):
                xt[32 * k:32 * k + 32, TC * c:TC * (c + 1)] = \
                    blk[:, :, k].T.astype(f16)
        sq = seqs[B * m:B * (m + 1)]                          # (32, NS)
        wt = np.zeros((128, (NS + 1) * 32), f16)
        for s in range(1, NS + 1):
            k = sq[:, s - 1]                                  # (32,)
            wt[32 * k + np.arange(B), 32 * s + np.arange(B)] = 1.0
        in_maps.append({"xtab": xt, "wtab": wt, "mats": mats,
                        "sched": sched, "iot": iot})
    return in_maps


def host_post(x, seqlens, outs, rzs):
    BETA = beta_schedule()
    x = np.asarray(x)
    seqlens = np.asarray(seqlens)
    loss = np.zeros((NB, 1), np.float32)
    for m in range(M):
        raw = outs[m][:, 0:NS + 1].astype(np.float64)         # (32, 501)
        rzv = rzs[m][0:32, :].astype(np.float64)              # rz per batch,epoch
        lnrz = np.zeros((32, SIG))
        for ep in range(1, NEP):
            sg = ep * R
            if sg < SIG:
                lnrz[:, sg] = np.log(np.maximum(rzv[:, ep], 1e-300))
        lnrz_cum = np.cumsum(lnrz, axis=1)
        C2000 = x[:, B * m:B * (m + 1), 4].sum(axis=0, dtype=np.float64)
        fwd = np.zeros((32, NS + 1))
        for s in range(NS + 1):
            sig = s + SKEW * (NC - 1)
            ep = sig // R
            corr = TC * BETA[:, ep].sum() - lnrz_cum[:, sig]
            fwd[:, s] = np.log(np.maximum(raw[:, s], 1e-300)) + corr + C2000
        sl = seqlens[B * m:B * (m + 1)].astype(np.int64)
        loss[B * m:B * (m + 1), 0] = \
            (-np.take_along_axis(fwd, sl[:, None], axis=1) / NT)[:, 0]
    return loss.astype(np.float32)


def kernel(x, seqs, seqlens):
    nc, names = build_module()
    in_maps = host_prep(x, seqs)
    res = run_bass_kernel_spmd(nc, in_maps, list(range(M)))
    outs = [res.results[m]["outst"] for m in range(M)]
    rzs = [res.results[m]["rzst"] for m in range(M)]
    return host_post(x, seqlens, outs, rzs)


# revision 9
# speedup vs baseline: 1.0840x; 1.0014x over previous
"""CTC forward-loss kernel for Trainium2, 8 cores data-parallel (32 batch/core).

v2 architecture (validated against a numpy prototype):
  Layout: 128 partitions = 4 t-chunks x 32 batches; free axis = t within
  chunk (500).  Skew-2 wavefront over columns s: chunk c scans column
  s = sigma - 2c at step sigma (507 steps).

  Per sigma:
    PE    4 select-matmuls W_s.T @ X_c -> PSUM produce the move coefficients
          b[t] = E[t, seqs[s-1]] for each chunk (one-hot W, fp16 tables),
          issued 2 sigmas ahead; plus a tiny halo-shift matmul.
    DVE   u = (b * d) * prv   (scalar_tensor_tensor from PSUM)
          cur[1:] = scan(d * y_prev + u)  (tensor_tensor_scan, fp32 state)
    ACT   init-slot copy (halo -> cur[:,0]) and chunk-3 output staging.

  Numerics: stored values are G * exp(-beta_c(epoch) * t_local) * renorm,
  with a compile-time beta schedule (binomial-slope model) and a shared
  per-batch renorm every R=8 sigmas measured from the full-buffer sum.
  Epoch transitions re-ramp the state on device (ACT exp of iota).  The
  host replays the schedule + logged renorm factors to undo all scales.
"""

import math
import numpy as np

import concourse.bass as bass
import concourse.tile as tile
import concourse.mybir as mybir
from concourse.bass_utils import run_bass_kernel_spmd

NT, NB, NF, NS = 2000, 256, 5, 500
M = 8
B = NB // M           # 32 local batches
NC, TC = 4, 500
SKEW = 2
SIG = NS + SKEW * (NC - 1) + 1          # 507 wavefront steps
R = 8                                    # renorm/epoch cadence
NEP = 80                                 # schedule table width (>= SIG//R + 2)
LA = 2                                   # select lookahead (psum bufs = 4)
ZTINY = 1e-30
F32 = mybir.dt.float32
F16 = mybir.dt.float16
AL = mybir.AluOpType
AF = mybir.ActivationFunctionType
AX = mybir.AxisListType

_CACHE = {}


# ---------------------------------------------------------------- schedule --

def _lnC(n, k):
    if k < 0 or k > n:
        return -math.inf
    return (math.lgamma(n + 1) - math.lgamma(k + 1) - math.lgamma(n - k + 1))


def _beta_model(c, s):
    t0, t1 = 500 * c, 500 * (c + 1)
    s = min(s, t1 - 64)
    tlo = max(t0, s + 1)
    if t1 <= tlo + 1:
        return 0.0
    num = _lnC(t1, s) - _lnC(tlo, s)
    if not math.isfinite(num):
        return 0.0
    return num / (t1 - tlo)


def beta_schedule():
    """BETA[c, ep], monotone non-decreasing per chunk."""
    n_ep = NEP
    BETA = np.zeros((NC, n_ep))
    for c in range(NC):
        prev = 0.0
        for ep in range(n_ep):
            smid = min(max(ep * R + R // 2 - SKEW * c, 0), NS)
            b = max(_beta_model(c, smid), prev)
            BETA[c, ep] = b
            prev = b
    return BETA


# ---------------------------------------------------------------- module ----

def _split_multi_waits(nc, max_inline=1):
    """walrus allows few fused sem-waits per instruction; hoist extras onto
    EventSemaphore carriers on the same engine stream."""
    f = nc.m.functions[0]
    for bb in f.blocks:
        out = []
        changed = False
        for inst in bb.instructions:
            si = inst.sync_info
            waits = list(si.on_wait) if si is not None and si.on_wait else []
            if isinstance(inst, mybir.InstEventSemaphore) and len(waits) > 2:
                keep, extra = waits[:2], waits[2:]
            elif not isinstance(inst, mybir.InstEventSemaphore) \
                    and len(waits) > max_inline:
                keep, extra = waits[:max_inline], waits[max_inline:]
            else:
                keep, extra = waits, []
            if extra:
                for j in range(0, len(extra), 2):
                    evt = mybir.InstEventSemaphore(
                        name=f"evw{j}_{inst.name}", ins=[], outs=[])
                    evt.engine = inst.engine
                    evt.sync_info = mybir.SyncInfo(
                        on_wait=extra[j:j + 2], on_update=[])
                    out.append(evt)
                inst.sync_info = mybir.SyncInfo(
                    on_wait=keep, on_update=list(si.on_update))
                changed = True
            out.append(inst)
        if changed:
            bb.instructions = out


def build_module(split_waits=True):
    key = ("nc", split_waits)
    if key in _CACHE:
        return _CACHE[key], _CACHE["names"]
    nc = bass.Bass(debug=False)

    xtab_d = nc.dram_tensor("xtab", [128, NC * TC], F16, kind="ExternalInput")
    wtab_d = nc.dram_tensor("wtab", [128, (NS + 1) * 32], F16, kind="ExternalInput")
    mats_d = nc.dram_tensor("mats", [128, 256], F32, kind="ExternalInput")
    sched_d = nc.dram_tensor("sched", [128, 4 * NEP], F32, kind="ExternalInput")
    iot_d = nc.dram_tensor("iot", [128, 512], F32, kind="ExternalInput")
    out_d = nc.dram_tensor("outst", [32, 512], F32, kind="ExternalOutput")
    rz_d = nc.dram_tensor("rzst", [128, NEP], F32, kind="ExternalOutput")

    with tile.TileContext(nc) as tc, \
         tc.tile_pool(name="state", bufs=1) as st:
        xtab = st.tile([128, NC * TC], F16, tag="xtab", name="xtab_sb")
        wtab = st.tile([128, (NS + 1) * 32], F16, tag="wtab", name="wtab_sb")
        mats = st.tile([128, 256], F32, tag="mats", name="mats_sb")
        sched = st.tile([128, 4 * NEP], F32, tag="sched", name="sched_sb")
        iot = st.tile([128, 512], F32, tag="iot", name="iot_sb")
        bufA = st.tile([128, TC + 1], F32, tag="bufA", name="bufA")
        bufB = st.tile([128, TC + 1], F32, tag="bufB", name="bufB")
        u = st.tile([128, TC + 1], F32, tag="u", name="u")
        d0row2 = [st.tile([128, TC], F32, tag=f"d0r{i}", name=f"d0r{i}")
                  for i in range(2)]
        ramp = st.tile([128, TC + 1], F32, tag="ramp", name="ramp")
        ones = st.tile([128, TC + 1], F32, tag="ones", name="ones")
        outst = st.tile([128, 512], F32, tag="outst", name="outst")
        rzst = st.tile([128, NEP], F32, tag="rzst", name="rzst")
        zred = st.tile([128, 1], F32, tag="zred", name="zred")
        zcl = st.tile([128, 1], F32, tag="zcl", name="zcl")
        haloSB = st.tile([128, 2], F32, tag="haloSB", name="haloSB")

        rz = st.tile([128, 1], F32, tag="rz", name="rz")
        vsc = st.tile([128, 1], F32, tag="vsc", name="vsc")

        nc.sync.dma_start(out=xtab[:, :], in_=xtab_d.ap()[:, :])
        nc.sync.dma_start(out=wtab[:, 0:2048], in_=wtab_d.ap()[:, 0:2048])
        nc.sync.dma_start(out=wtab[:, 2048:], in_=wtab_d.ap()[:, 2048:])
        nc.sync.dma_start(out=mats[:, :], in_=mats_d.ap()[:, :])
        nc.sync.dma_start(out=sched[:, :], in_=sched_d.ap()[:, :])
        nc.sync.dma_start(out=iot[:, :], in_=iot_d.ap()[:, :])

        DT = sched[:, 0 * NEP:1 * NEP]       # exp(-beta_c(ep))
        DBN = sched[:, 1 * NEP:2 * NEP]      # -(beta_c(ep) - beta_c(ep-1))
        DBI = sched[:, 2 * NEP:3 * NEP]      # -TC * sum_{c'<c} dbeta_{c'}(ep)
        FXB = sched[:, 3 * NEP:4 * NEP]      # exp(-TC*sum_{c'<=c-1} dbeta(ep))
        PERM = mats[:, 0:128]
        GSUM = mats[:, 128:256]

        for t in (bufA, bufB, u, outst, rzst, haloSB):
            nc.vector.memset(t[:, :], 0.0)
        nc.vector.memset(ones[:, :], 1.0)
        # touch DMA'd tiles once (wait-slot pressure)
        scr = st.tile([128, 1], F32, tag="scr", name="scr")
        nc.vector.tensor_copy(scr[:, 0:1], mats[:, 0:1])
        nc.vector.tensor_copy(scr[:, 0:1], sched[:, 0:1])
        nc.vector.tensor_copy(scr[:, 0:1], iot[:, 0:1])

        # initial d0row for epoch 0
        nc.scalar.mul(d0row2[0][:, 0:TC], ones[:, 0:TC], DT[:, 0:1])

        with tc.tile_pool(name="bsel", bufs=4, space="PSUM") as bp, \
             tc.tile_pool(name="zs", bufs=2, space="PSUM") as zp:
            bps = [None] * (SIG + LA)

            def issue_selects(sL):
                bt = bp.tile([128, 512], F32, tag="bsel")
                bps[sL] = bt
                for c in range(NC):
                    s = sL - SKEW * c
                    slot = s if 1 <= s <= NS else 0
                    nc.tensor.matmul(
                        bt[32 * c:32 * c + 32, 0:TC],
                        wtab[:, 32 * slot:32 * slot + 32],
                        xtab[:, TC * c:TC * (c + 1)],
                        start=True, stop=True, tile_position=(0, 32 * c))

            for sL in range(LA):
                issue_selects(sL)

            for sig in range(SIG):
                cur, prv = (bufA, bufB) if sig % 2 == 0 else (bufB, bufA)
                ep = sig // R
                event = (sig % R == 0 and sig > 0)
                if event:
                    nc.vector.tensor_mul(prv[:, 0:TC + 1], prv[:, 0:TC + 1],
                                         ramp[:, 0:TC + 1])
                    # shared renorm from full-buffer sum
                    nc.vector.tensor_reduce(zred[:, 0:1],
                                            prv[:, 0:TC + 1:4],
                                            AX.X, AL.add)
                    zsp = zp.tile([128, 1], F32, tag="zs")
                    nc.tensor.matmul(zsp[:, 0:1], GSUM, zred[:, 0:1],
                                     start=True, stop=True)
                    nc.vector.tensor_scalar_max(zcl[:, 0:1], zsp[:, 0:1], ZTINY)
                    nc.vector.reciprocal(rz[:, 0:1], zcl[:, 0:1])
                    nc.scalar.mul(prv[:, 0:TC + 1], prv[:, 0:TC + 1], rz[:, 0:1])
                    nc.scalar.copy(rzst[:, ep:ep + 1], rz[:, 0:1])
                    nc.scalar.mul(vsc[:, 0:1], FXB[:, ep:ep + 1], rz[:, 0:1])

                # select lookahead
                if sig + LA < SIG:
                    issue_selects(sig + LA)
                # prefetch next epoch's ramp and drain row (ACT, early)
                if (sig + 2) % R == 0 and sig + 2 < SIG:
                    epn = (sig + 2) // R
                    nc.scalar.activation(ramp[:, 0:TC + 1], iot[:, 0:TC + 1],
                                         AF.Exp, scale=DBN[:, epn:epn + 1],
                                         bias=DBI[:, epn:epn + 1])
                    nc.scalar.mul(d0row2[epn % 2][:, 0:TC], ones[:, 0:TC],
                                  DT[:, epn:epn + 1])


                # init slot: halo (written at sig-2) with event fixes
                if sig == 0:
                    nc.vector.memset(cur[:, 0:1], 0.0)
                    nc.vector.memset(cur[0:32, 0:1], 1.0)
                else:
                    hsrc = haloSB[:, sig % 2:sig % 2 + 1]
                    if sig % R in (0, 1) and sig >= R:
                        nc.scalar.activation(cur[:, 0:1], hsrc, AF.Copy,
                                             scale=vsc[:, 0:1])
                    else:
                        nc.scalar.activation(cur[:, 0:1], hsrc, AF.Copy)

                # u = (b * d) * prv ; slot 0 of u stays 0
                nc.vector.scalar_tensor_tensor(
                    u[:, 0:TC], bps[sig][:, 0:TC], DT[:, ep:ep + 1],
                    prv[:, 0:TC], AL.mult, AL.mult)
                # scan
                nc.vector.tensor_tensor_scan(
                    cur[:, 1:TC + 1], d0row2[ep % 2][:, 0:TC], u[:, 0:TC],
                    cur[:, 0:1], AL.mult, AL.add)

                # halo extraction for sig+2 (partition-shift by +32 via DMA)
                if sig + 2 < SIG:
                    nc.sync.dma_start(
                        out=haloSB[32:128, sig % 2:sig % 2 + 1],
                        in_=cur[0:96, TC:TC + 1])

                # chunk-3 output staging
                s3 = sig - SKEW * (NC - 1)
                if 0 <= s3 <= NS:
                    nc.scalar.copy(outst[96:128, s3:s3 + 1],
                                   cur[96:128, TC:TC + 1])

        nc.sync.dma_start(out=out_d.ap()[:, :], in_=outst[96:128, 0:512])
        nc.sync.dma_start(out=rz_d.ap()[:, :], in_=rzst[:, :])

    if split_waits:
        _split_multi_waits(nc)

    _CACHE[key] = nc
    _CACHE["names"] = dict(ins=["xtab", "wtab", "mats", "sched", "iot"],
                           out=["outst", "rzst"])
    return nc, _CACHE["names"]


# ---------------------------------------------------------------- host ------

def host_prep(x, seqs):
    f32, f16 = np.float32, np.float16
    BETA = beta_schedule()
    x = np.asarray(x)
    seqs = np.asarray(seqs)

    # schedule tables, per partition p = 32c + b
    dt_t = np.zeros((128, NEP), f32)
    dbn_t = np.zeros((128, NEP), f32)
    dbi_t = np.zeros((128, NEP), f32)
    fxb_t = np.ones((128, NEP), f32)
    for ep in range(NEP):
        db = BETA[:, ep] - (BETA[:, ep - 1] if ep > 0 else BETA[:, 0])
        cum = np.concatenate([[0.0], np.cumsum(db)[:-1]])    # sum_{c'<c}
        dsum = np.cumsum(db)                                  # sum_{c'<=c}
        for c in range(NC):
            p = slice(32 * c, 32 * c + 32)
            dt_t[p, ep] = np.exp(-BETA[c, ep])
            dbn_t[p, ep] = -db[c]
            dbi_t[p, ep] = -TC * cum[c]
            fxb_t[p, ep] = np.exp(-TC * dsum[c - 1]) if c >= 1 else 1.0
    sched = np.concatenate([dt_t, dbn_t, dbi_t, fxb_t], axis=1)

    iot = np.broadcast_to(np.arange(512, dtype=f32), (128, 512)).copy()

    mats = np.zeros((128, 256), f32)
    for q in range(96):
        mats[q, 32 + q] = 1.0            # perm: out[p] = in[p-32]
    for pi in range(128):
        for c in range(NC):
            mats[pi, 128 + ((pi % 32) + 32 * c) % 128] = 0.0  # placeholder
    # gsum: out[po] = sum_c in[32c + (po%32)]
    gs = np.zeros((128, 128), f32)
    for po in range(128):
        for c in range(NC):
            gs[32 * c + (po % 32), po] = 1.0
    mats[:, 128:256] = gs

    in_maps = []
    for m in range(M):
        xb = x[:, B * m:B * (m + 1), :].astype(f32)           # (NT, 32, 5)
        E = np.exp(xb[:, :, :4] - xb[:, :, 4:5])              # (NT, 32, 4)
        xt = np.empty((128, NC * TC), f16)
        for c in range(NC):
            blk = E[500 * c:500 * (c + 1)]                    # (500, 32, 4)
            for k in range(4):
                xt[32 * k:32 * k + 32, TC * c:TC * (c + 1)] = \
                    blk[:, :, k].T.astype(f16)
        sq = seqs[B * m:B * (m + 1)]                          # (32, NS)
        wt = np.zeros((128, (NS + 1) * 32), f16)
        for s in range(1, NS + 1):
            k = sq[:, s - 1]                                  # (32,)
            wt[32 * k + np.arange(B), 32 * s + np.arange(B)] = 1.0
        in_maps.append({"xtab": xt, "wtab": wt, "mats": mats,
                        "sched": sched, "iot": iot})
    return in_maps


def host_post(x, seqlens, outs, rzs):
    BETA = beta_schedule()
    x = np.asarray(x)
    seqlens = np.asarray(seqlens)
    loss = np.zeros((NB, 1), np.float32)
    for m in range(M):
        raw = outs[m][:, 0:NS + 1].astype(np.float64)         # (32, 501)
        rzv = rzs[m][0:32, :].astype(np.float64)              # rz per batch,epoch
        lnrz = np.zeros((32, SIG))
        for ep in range(1, NEP):
            sg = ep * R
            if sg < SIG:
                lnrz[:, sg] = np.log(np.maximum(rzv[:, ep], 1e-300))
        lnrz_cum = np.cumsum(lnrz, axis=1)
        C2000 = x[:, B * m:B * (m + 1), 4].sum(axis=0, dtype=np.float64)
        fwd = np.zeros((32, NS + 1))
        for s in range(NS + 1):
            sig = s + SKEW * (NC - 1)
            ep = sig // R
            corr = TC * BETA[:, ep].sum() - lnrz_cum[:, sig]
            fwd[:, s] = np.log(np.maximum(raw[:, s], 1e-300)) + corr + C2000
        sl = seqlens[B * m:B * (m + 1)].astype(np.int64)
        loss[B * m:B * (m + 1), 0] = \
            (-np.take_along_axis(fwd, sl[:, None], axis=1) / NT)[:, 0]
    return loss.astype(np.float32)


def kernel(x, seqs, seqlens):
    nc, names = build_module()
    in_maps = host_prep(x, seqs)
    res = run_bass_kernel_spmd(nc, in_maps, list(range(M)))
    outs = [res.results[m]["outst"] for m in range(M)]
    rzs = [res.results[m]["rzst"] for m in range(M)]
    return host_post(x, seqlens, outs, rzs)
